# revision 45
# baseline (speedup 1.0000x reference)
"""BERT+CRF loss (torchcrf-style, reduction=sum) on 8 Trainium2 NeuronCores.

Strategy (pure data parallel, batch sharded 8 ways, 8 sequences per core):
  emissions^T = W^T @ X^T on TensorE (X pre-transposed + cast to bf16/fp8 on
  host).  Raw emissions^T [9,S] are downloaded (bf16) and the CRF numerator
  (gold-path score) is computed on host.  CRF forward recurrence in exp space:
      v_t = (v_{t-1}^T expT) * E_t,  E_t = exp(em_t)
  Adjacent steps are paired into 9x9 transfer matrices
      B_p[i,j] = sum_k expT[i,k] E_{2p+1}[k] expT[k,j] E_{2p+2}[j]
  computed on TensorE as  outer(E_a, E_b) [81] x G4 [81,81]  (G4 is a host
  constant built from exp(trans)).  Each sequence's 255 pair matrices are
  split into 16 chunks of 16; a chunk-parallel matrix product runs on
  VectorE in bf16 with 128 partitions = 8 batches x 16 chunks, 15 steps
  (state initialized from step 0), periodic max-normalization for range
  safety.  Pair matrices reach the chunk layout via direct SBUF->SBUF DMA
  (no DRAM bounce).  Host combines the 16 chunk matrices per sequence
  (O(B*16*81) f64) and adds the label-indexed numerator terms.
"""

import sys

if "/opt/trn_rl_repo" not in sys.path:
    sys.path.insert(0, "/opt/trn_rl_repo")

import numpy as np

B, S, H, L = 64, 512, 768, 9
NCORES = 8
BPC = B // NCORES          # sequences per core
LL = L * L                 # 81
NPAIR = 256                # pair slots per sequence (255 real + 1 identity)
NQUAD = 128                # quad matrices per sequence (pairs merged on-chip)
NCHUNK = 16                # chunks per sequence
SPC = NQUAD // NCHUNK      # quad-steps per chunk = 8
HC = H // 128              # 6 contraction chunks of 128
NORM_STEPS = (2, 5)        # recurrence steps after which we renormalize
NNORM = len(NORM_STEPS)
EM_FP8 = True              # emissions matmul in fp8e4 DoubleRow (W scaled)
WSCALE = 64.0              # fp8 W prescale (undone in exp + host)

_CACHE = {}


def _build_bass():
    import concourse.bass as bass
    import concourse.bacc as bacc
    import concourse.mybir as mybir
    import concourse.tile as tile
    from contextlib import ExitStack

    f32 = mybir.dt.float32
    bf16 = mybir.dt.bfloat16
    em_dt = mybir.dt.float8e4 if EM_FP8 else bf16
    Alu = mybir.AluOpType
    Act = mybir.ActivationFunctionType
    Ax = mybir.AxisListType

    nc = bacc.Bacc()

    # ---- I/O (all host-prearranged, dense layouts) ----
    # weight rows padded to 16 elems/chunk: DoubleRow needs dual-row step%16==0
    WP = 16 if EM_FP8 else L
    xT_d = nc.dram_tensor("xT", [BPC, 128, HC * S], em_dt, kind="ExternalInput")
    w_d = nc.dram_tensor("Wt", [128, HC * WP], em_dt, kind="ExternalInput")
    g4r_d = nc.dram_tensor("G4R", [LL, LL], bf16, kind="ExternalInput")
    g4c_d = nc.dram_tensor("G4C", [LL, LL], bf16, kind="ExternalInput")
    ra_d = nc.dram_tensor("Ra", [L, LL], bf16, kind="ExternalInput")
    rb_d = nc.dram_tensor("Rb", [L, LL], bf16, kind="ExternalInput")
    id_d = nc.dram_tensor("Id128", [128, LL], bf16, kind="ExternalInput")

    em_out = nc.dram_tensor("em_out", [BPC, L, S], bf16, kind="ExternalOutput")
    s_out = nc.dram_tensor("S_out", [128, LL], bf16, kind="ExternalOutput")
    m_out = nc.dram_tensor("m_out", [128, NNORM], f32, kind="ExternalOutput")

    with ExitStack() as ctx:
        tc = ctx.enter_context(tile.TileContext(nc))
        const = ctx.enter_context(tc.tile_pool(name="const", bufs=1))
        xpool = ctx.enter_context(tc.tile_pool(name="x", bufs=BPC))
        epool = ctx.enter_context(tc.tile_pool(name="e", bufs=3))
        empool = ctx.enter_context(tc.tile_pool(name="em", bufs=3))
        spool = ctx.enter_context(tc.tile_pool(name="sm", bufs=3))
        bpool = ctx.enter_context(tc.tile_pool(name="bsb", bufs=3))
        qpool = ctx.enter_context(tc.tile_pool(name="quad", bufs=3))
        rpool = ctx.enter_context(tc.tile_pool(name="rec", bufs=1))
        dpool = ctx.enter_context(tc.tile_pool(name="dram", bufs=1, space="DRAM"))
        ps_em = ctx.enter_context(tc.tile_pool(name="psem", bufs=4, space="PSUM"))
        ps_rep = ctx.enter_context(tc.tile_pool(name="psrep", bufs=1, space="PSUM"))
        ps_b = ctx.enter_context(tc.tile_pool(name="psb", bufs=2, space="PSUM"))

        # ---- constants into SBUF (already target dtype on host) ----
        w_sb = const.tile([128, HC * WP], em_dt)
        nc.gpsimd.dma_start(w_sb[:], w_d[:])
        g4r_sb = const.tile([LL, LL], bf16)
        nc.gpsimd.dma_start(g4r_sb[:], g4r_d[:])
        g4c_sb = const.tile([LL, LL], bf16)
        nc.gpsimd.dma_start(g4c_sb[:], g4c_d[:])
        ra_sb = const.tile([L, LL], bf16)
        nc.gpsimd.dma_start(ra_sb[:], ra_d[:])
        rb_sb = const.tile([L, LL], bf16)
        nc.gpsimd.dma_start(rb_sb[:], rb_d[:])
        # ---- persistent recurrence state ----
        s_tile = rpool.tile([128, LL], bf16)           # chunk-product state
        bc_tile = rpool.tile([128, SPC * LL], bf16)    # quad matrices, chunk layout
        tmp729 = rpool.tile([128, L * L * L], bf16)
        mvals = rpool.tile([128, NNORM], f32)          # applied reciprocal scales

        # DRAM bounce for the quad-layout -> chunk-layout regroup (bf16)
        b_all = dpool.tile([BPC, NQUAD, LL], bf16)

        SP = S + 3  # e_sb column pad (pair col index reaches S; keep 4B align)

        # prefetch all X up front so TensorE never waits on HBM
        xts = []
        for b in range(BPC):
            xt = xpool.tile([128, HC * S], em_dt)
            if b == 0:
                # split so the first matmul can start after half the DMA
                nc.sync.dma_start(xt[:, 0 : 2 * S], xT_d[b, :, 0 : 2 * S])
                nc.sync.dma_start(xt[:, 2 * S : 6 * S], xT_d[b, :, 2 * S : 6 * S])
            else:
                nc.sync.dma_start(xt[:], xT_d[b])
            xts.append(xt)

        def emissions(b, e2, q):
            """Emissions matmul + em download + exp for sequence b.
            Writes exp(em) into half q of the shared pair tile e2."""
            xt = xts[b]
            em_ps = ps_em.tile([L, S], f32)
            if EM_FP8:
                for c in range(HC // 2):
                    nc.tensor.matmul(
                        em_ps[:],
                        w_sb[:, 2 * c * WP : (2 * c + 2) * WP].rearrange(
                            "k (t l) -> k t l", t=2
                        )[:, :, 0:L],
                        xt[:, 2 * c * S : (2 * c + 2) * S].rearrange(
                            "k (t s) -> k t s", t=2
                        ),
                        start=(c == 0),
                        stop=(c == HC // 2 - 1),
                        perf_mode=mybir.MatmulPerfMode.DoubleRow,
                    )
            else:
                for c in range(HC):
                    nc.tensor.matmul(
                        em_ps[:],
                        w_sb[:, c * L : (c + 1) * L],
                        xt[:, c * S : (c + 1) * S],
                        start=(c == 0),
                        stop=(c == HC - 1),
                    )

            # raw emissions download (host computes numerator + v0 + tail)
            em_bf = empool.tile([L, S], bf16)
            nc.scalar.copy(em_bf[:], em_ps[:])
            nc.gpsimd.dma_start(em_out[b], em_bf[:])

            # E = exp(em) in bf16, with a zero column at index S
            nc.vector.memset(e2[:, q, S:SP], 0.0)
            nc.scalar.activation(
                e2[:, q, 0:S], em_ps[:], Act.Exp,
                scale=1.0 / WSCALE if EM_FP8 else 1.0,
            )

        def pair_block(b, e2):
            """Pair matrices for sequences b, b+1 (one batched replication)."""
            # both sequences' E columns in one moving operand [9, 2, 256]
            pstride = e2[:].ap[0][0]
            off = e2[:].offset
            ea_ap = bass.AP(
                e2.tensor, off + 1, [[pstride, L], [SP, 2], [2, NPAIR]]
            )
            eb_ap = bass.AP(
                e2.tensor, off + 2, [[pstride, L], [SP, 2], [2, NPAIR]]
            )
            earep = ps_rep.tile([LL, 2 * NPAIR], f32)
            nc.tensor.matmul(earep[:], ra_sb[:], ea_ap, start=True, stop=True)
            ebrep = ps_rep.tile([LL, 2 * NPAIR], f32)
            nc.tensor.matmul(ebrep[:], rb_sb[:], eb_ap, start=True, stop=True)
            # one PSUM->SBUF copy, then outer = Ea*Eb (one PSUM read allowed)
            ebcp = spool.tile([LL, 2 * NPAIR], bf16)
            nc.scalar.copy(ebcp[:], ebrep[:])
            outer = spool.tile([LL, 2 * NPAIR], bf16)
            nc.vector.tensor_mul(outer[:], earep[:], ebcp[:])

            o_t = outer.tensor
            o_off = outer[:].offset
            o_ps = outer[:].ap[0][0]
            last = b == BPC - 2
            for q in range(2):          # sequence within the pair
                tail = last and q == 1
                bsb = bpool.tile([128, 2 * LL], bf16)
                # pair 255 (odd slot of partition 127) is the zero filler; it
                # must be identity so quad 127 = B_254.  Disjoint region, so
                # this DMA issues immediately and never blocks the merge.
                nc.gpsimd.dma_start(bsb[127:128, LL : 2 * LL], id_d[0:1, :])
                for h in range(2):      # h=0: even pairs (row-major B),
                    bp = ps_b.tile([128, LL], f32)   # h=1: odd (col-major)
                    ocols = bass.AP(
                        o_t, o_off + q * NPAIR + h, [[o_ps, LL], [2, 128]]
                    )
                    nc.tensor.matmul(
                        bp[:], ocols, (g4r_sb if h == 0 else g4c_sb)[:],
                        start=True, stop=True,
                    )
                    nr = 128 if h == 0 else 127
                    nc.vector.tensor_copy(
                        bsb[0:nr, h * LL : (h + 1) * LL], bp[0:nr, :]
                    )
                # quad merge: Q_p = B_{2p} @ B_{2p+1}, emitted col-major
                in0 = (
                    bsb[:, 0:LL].rearrange("p (i k) -> p i k", i=L)
                    .unsqueeze(1).broadcast_to([128, L, L, L])
                )
                in1 = (
                    bsb[:, LL : 2 * LL].rearrange("p (j k) -> p j k", j=L)
                    .unsqueeze(2).broadcast_to([128, L, L, L])
                )
                t3 = tmp729[:].rearrange("p (j i k) -> p j i k", j=L, i=L)
                nc.vector.tensor_tensor(out=t3, in0=in0, in1=in1, op=Alu.mult)
                qsb = qpool.tile([128, LL], bf16)
                with nc.allow_low_precision(reason="host chains in f64"):
                    nc.vector.tensor_reduce(
                        out=qsb[:], in_=t3, axis=Ax.X, op=Alu.add
                    )
                # bounce: quad rows out, chunk-layout read back
                rd = b_all[b + q].rearrange("(c s) j -> c (s j)", c=NCHUNK)
                rows = bc_tile[16 * (b + q) : 16 * (b + q + 1), :]
                nc.scalar.dma_start(b_all[b + q], qsb[:])
                if tail:
                    # early slots first so the recurrence can start sooner
                    nc.sync.dma_start(rows[:, 0 : 3 * LL], rd[:, 0 : 3 * LL])
                    nc.sync.dma_start(rows[:, 3 * LL :], rd[:, 3 * LL :])
                elif last:
                    nc.sync.dma_start(rows, rd)
                else:
                    nc.gpsimd.dma_start(rows, rd)

        # run emissions well ahead of the pair blocks: TensorE stays dense
        # (no LOW-p-state restarts) and rep/pair matmuls never wait on exp
        e2s = {}
        for b in range(BPC):
            if b % 2 == 0:
                e2 = epool.tile([L, 2, SP], bf16, name=f"e2_{b}")
                e2s[b] = e2
            emissions(b, e2s[b - b % 2], b % 2)
            if b == 3:
                pair_block(0, e2s[0])
            elif b == 5:
                pair_block(2, e2s[2])
            elif b == 7:
                pair_block(4, e2s[4])
        pair_block(6, e2s[6])

        # ---- chunk-parallel matrix recurrence: S <- S @ Q_s (bf16) ----
        # init: S = Q_0 (stored col-major; transpose-copy to row-major)
        nc.vector.tensor_copy(
            s_tile[:].rearrange("p (i j) -> p i j", i=L),
            bc_tile[:, 0:LL].rearrange("p (j i) -> p i j", j=L),
        )
        ncol = 0
        for s in range(1, SPC):
            bs = bc_tile[:, s * LL : (s + 1) * LL]
            in0 = (
                s_tile[:].rearrange("p (i k) -> p i k", i=L)
                .unsqueeze(2).broadcast_to([128, L, L, L])
            )
            # bc stores B^T (column-major B): inner k is contiguous
            in1 = (
                bs.rearrange("p (j k) -> p j k", j=L)
                .unsqueeze(1).broadcast_to([128, L, L, L])
            )
            t3 = tmp729[:].rearrange("p (i j k) -> p i j k", i=L, j=L)
            nc.vector.tensor_tensor(out=t3, in0=in0, in1=in1, op=Alu.mult)
            with nc.allow_low_precision(reason="9-term sums; host chains in f64"):
                nc.vector.tensor_reduce(
                    out=s_tile[:], in_=t3, axis=Ax.X, op=Alu.add
                )
            if s in NORM_STEPS:
                mc = spool.tile([128, 1], f32)
                nc.vector.reduce_max(mc[:], s_tile[:], axis=Ax.X)
                rec = mvals[:, ncol : ncol + 1]
                ncol += 1
                nc.vector.reciprocal(rec, mc[:])
                nc.vector.tensor_scalar_mul(s_tile[:], s_tile[:], rec)

        nc.sync.dma_start(s_out[:], s_tile[:])
        nc.sync.dma_start(m_out[:], mvals[:])

    if not nc.is_finalized():
        nc.finalize()
    return nc


def _get_nc():
    if "nc" not in _CACHE:
        _CACHE["nc"] = _build_bass()
    return _CACHE["nc"]


def _host_consts(trans):
    import ml_dtypes

    bf = ml_dtypes.bfloat16
    expT = np.exp(trans.astype(np.float64)).astype(np.float32)  # [9,9]
    k_idx = np.arange(LL) // L   # row index of the 81-flat (k, jb)
    jb_idx = np.arange(LL) % L
    i_idx = np.arange(LL) // L   # col index of the 81-flat (i, j)
    j_idx = np.arange(LL) % L
    # G4[(k,jb),(i,j)] = expT[i,k] * expT[k,j] * (j == jb)
    g4 = (
        expT[np.ix_(i_idx, k_idx)].T
        * expT[np.ix_(k_idx, j_idx)]
        * (j_idx[None, :] == jb_idx[:, None])
    ).astype(np.float32)
    g4r = np.ascontiguousarray(g4).astype(bf)       # row-major B (even pairs)
    # column-major B (odd pairs): contiguous reads in the quad merge
    g4c = np.ascontiguousarray(
        g4.reshape(LL, L, L).swapaxes(1, 2).reshape(LL, LL)
    ).astype(bf)
    ra = (k_idx[None, :] == np.arange(L)[:, None]).astype(bf)   # [9,81]
    rb = (jb_idx[None, :] == np.arange(L)[:, None]).astype(bf)  # [9,81]
    id128 = np.tile(
        np.eye(L, dtype=np.float32).reshape(1, LL), (128, 1)
    ).astype(bf)
    return expT, g4r, g4c, ra, rb, id128


def _numpy_reference(hs, mask, labels, W, bb, st, en, tr):
    # general fallback (only used when attention_mask is not all ones)
    em = hs.astype(np.float64) @ W.astype(np.float64) + bb.astype(np.float64)
    maskb = mask.astype(bool)
    maskf = mask.astype(np.float64)
    em_tag = np.take_along_axis(em, labels[..., None], axis=-1)[..., 0]
    num = st.astype(np.float64)[labels[:, 0]] + em_tag[:, 0]
    trs = tr.astype(np.float64)[labels[:, :-1], labels[:, 1:]]
    num = num + np.sum((trs + em_tag[:, 1:]) * maskf[:, 1:], axis=1)
    last = mask.sum(axis=1).astype(np.int64) - 1
    num = num + en.astype(np.float64)[labels[np.arange(len(labels)), last]]
    alpha = st.astype(np.float64)[None, :] + em[:, 0]
    for t in range(1, em.shape[1]):
        x = alpha[:, :, None] + tr.astype(np.float64)[None, :, :] + em[:, t][:, None, :]
        m = x.max(axis=1, keepdims=True)
        nxt = np.log(np.exp(x - m).sum(axis=1)) + m[:, 0, :]
        alpha = np.where(maskb[:, t][:, None], nxt, alpha)
    x = alpha + en.astype(np.float64)[None, :]
    m = x.max(axis=1, keepdims=True)
    denom = np.log(np.exp(x - m).sum(axis=1)) + m[:, 0]
    return np.asarray((denom - num).sum(), dtype=np.float32)


def kernel(**inputs):
    import ml_dtypes
    from concourse import bass_utils

    hs = np.asarray(inputs["hidden_states"], dtype=np.float32)
    mask = np.asarray(inputs["attention_mask"])
    labels = np.asarray(inputs["labels"]).astype(np.int64)
    W = np.asarray(inputs["W"], dtype=np.float32)
    bb = np.asarray(inputs["b"], dtype=np.float32)
    st = np.asarray(inputs["start_trans"], dtype=np.float32)
    en = np.asarray(inputs["end_trans"], dtype=np.float32)
    tr = np.asarray(inputs["trans"], dtype=np.float32)

    if not np.all(mask == 1):
        return _numpy_reference(hs, mask, labels, W, bb, st, en, tr)

    em_np = ml_dtypes.float8_e4m3 if EM_FP8 else ml_dtypes.bfloat16
    expT, g4r, g4c, ra, rb, id128 = _host_consts(tr)

    # X^T in matmul layout: [B, 128, HC*S], partition k holds H rows c*128+k
    if EM_FP8:
        xc = hs.astype(em_np)
    else:
        xc = hs.astype(em_np)
    xT = np.ascontiguousarray(
        xc.reshape(B, S, HC, 128).transpose(0, 3, 2, 1)
    ).reshape(B, 128, HC * S)
    ws = (W * WSCALE) if EM_FP8 else W
    wT = np.ascontiguousarray(
        ws.reshape(HC, 128, L).transpose(1, 0, 2)
    ).astype(em_np)                                   # [128, HC, L]
    if EM_FP8:
        wp = np.zeros((128, HC, 16), dtype=em_np)
        wp[:, :, :L] = wT
        wT = wp
    wT = wT.reshape(128, -1)

    nc = _get_nc()
    in_maps = []
    for k in range(NCORES):
        sl = slice(k * BPC, (k + 1) * BPC)
        in_maps.append(
            {
                "xT": xT[sl],
                "Wt": wT,
                "G4R": g4r,
                "G4C": g4c,
                "Ra": ra,
                "Rb": rb,
                "Id128": id128,
            }
        )
    res = bass_utils.run_bass_kernel_spmd(nc, in_maps, list(range(NCORES)))
    _CACHE["last_results"] = res

    # ---- host combine (f64, tiny) ----
    expT64 = np.exp(tr.astype(np.float64))
    e_end = np.exp(en.astype(np.float64))
    st64 = st.astype(np.float64)
    bb64 = bb.astype(np.float64)
    en64 = en.astype(np.float64)
    tr64 = tr.astype(np.float64)
    total = 0.0
    for k in range(NCORES):
        r = res.results[k]
        em = r["em_out"].astype(np.float64)          # [BPC, 9, S]
        if EM_FP8:
            em = em / WSCALE
        Sf = r["S_out"].astype(np.float64).reshape(BPC, NCHUNK, L, L)
        mv = r["m_out"].astype(np.float64).reshape(BPC, NCHUNK, NNORM)
        for b in range(BPC):
            v = np.exp(em[b, :, 0] + st64 + bb64)    # v0
            logacc = -np.log(mv[b]).sum()            # undo applied scales
            for c in range(NCHUNK):
                v = v @ Sf[b, c]
                m = v.max()
                v /= m
                logacc += np.log(m)
            v = (v @ expT64) * np.exp(em[b, :, S - 1] + bb64)  # tail t = S-1
            total += np.log(v @ e_end) + logacc
        # numerator for this core's sequences (gold path score)
        lb = labels[k * BPC : (k + 1) * BPC]
        em_tag = np.take_along_axis(em, lb[:, None, :], axis=1)[:, 0, :]  # [BPC,S]
        total -= float(
            em_tag.sum()
            + st64[lb[:, 0]].sum()
            + en64[lb[:, -1]].sum()
            + tr64[lb[:, :-1], lb[:, 1:]].sum()
            + bb64[lb].sum()
        )
    return np.asarray(total, dtype=np.float32)


# revision 46
# speedup vs baseline: 1.0012x; 1.0012x over previous
"""BERT+CRF loss (torchcrf-style, reduction=sum) on 8 Trainium2 NeuronCores.

Strategy (pure data parallel, batch sharded 8 ways, 8 sequences per core):
  emissions^T = W^T @ X^T on TensorE (X pre-transposed + cast to bf16/fp8 on
  host).  Raw emissions^T [9,S] are downloaded (bf16) and the CRF numerator
  (gold-path score) is computed on host.  CRF forward recurrence in exp space:
      v_t = (v_{t-1}^T expT) * E_t,  E_t = exp(em_t)
  Adjacent steps are paired into 9x9 transfer matrices
      B_p[i,j] = sum_k expT[i,k] E_{2p+1}[k] expT[k,j] E_{2p+2}[j]
  computed on TensorE as  outer(E_a, E_b) [81] x G4 [81,81]  (G4 is a host
  constant built from exp(trans)).  Each sequence's 255 pair matrices are
  split into 16 chunks of 16; a chunk-parallel matrix product runs on
  VectorE in bf16 with 128 partitions = 8 batches x 16 chunks, 15 steps
  (state initialized from step 0), periodic max-normalization for range
  safety.  Pair matrices reach the chunk layout via direct SBUF->SBUF DMA
  (no DRAM bounce).  Host combines the 16 chunk matrices per sequence
  (O(B*16*81) f64) and adds the label-indexed numerator terms.
"""

import sys

if "/opt/trn_rl_repo" not in sys.path:
    sys.path.insert(0, "/opt/trn_rl_repo")

import numpy as np

B, S, H, L = 64, 512, 768, 9
NCORES = 8
BPC = B // NCORES          # sequences per core
LL = L * L                 # 81
NPAIR = 256                # pair slots per sequence (255 real + 1 identity)
NQUAD = 128                # quad matrices per sequence (pairs merged on-chip)
NCHUNK = 16                # chunks per sequence
SPC = NQUAD // NCHUNK      # quad-steps per chunk = 8
HC = H // 128              # 6 contraction chunks of 128
NORM_STEPS = (3,)          # recurrence steps after which we renormalize
NNORM = len(NORM_STEPS)
EM_FP8 = True              # emissions matmul in fp8e4 DoubleRow (W scaled)
WSCALE = 64.0              # fp8 W prescale (undone in exp + host)

_CACHE = {}


def _build_bass():
    import concourse.bass as bass
    import concourse.bacc as bacc
    import concourse.mybir as mybir
    import concourse.tile as tile
    from contextlib import ExitStack

    f32 = mybir.dt.float32
    bf16 = mybir.dt.bfloat16
    em_dt = mybir.dt.float8e4 if EM_FP8 else bf16
    Alu = mybir.AluOpType
    Act = mybir.ActivationFunctionType
    Ax = mybir.AxisListType

    nc = bacc.Bacc()

    # ---- I/O (all host-prearranged, dense layouts) ----
    # weight rows padded to 16 elems/chunk: DoubleRow needs dual-row step%16==0
    WP = 16 if EM_FP8 else L
    xT_d = nc.dram_tensor("xT", [BPC, 128, HC * S], em_dt, kind="ExternalInput")
    w_d = nc.dram_tensor("Wt", [128, HC * WP], em_dt, kind="ExternalInput")
    g4r_d = nc.dram_tensor("G4R", [LL, LL], bf16, kind="ExternalInput")
    g4c_d = nc.dram_tensor("G4C", [LL, LL], bf16, kind="ExternalInput")
    ra_d = nc.dram_tensor("Ra", [L, LL], bf16, kind="ExternalInput")
    rb_d = nc.dram_tensor("Rb", [L, LL], bf16, kind="ExternalInput")
    id_d = nc.dram_tensor("Id128", [128, LL], bf16, kind="ExternalInput")

    em_out = nc.dram_tensor("em_out", [BPC, L, S], bf16, kind="ExternalOutput")
    s_out = nc.dram_tensor("S_out", [128, LL], bf16, kind="ExternalOutput")
    m_out = nc.dram_tensor("m_out", [128, NNORM], f32, kind="ExternalOutput")

    with ExitStack() as ctx:
        tc = ctx.enter_context(tile.TileContext(nc))
        const = ctx.enter_context(tc.tile_pool(name="const", bufs=1))
        xpool = ctx.enter_context(tc.tile_pool(name="x", bufs=BPC))
        epool = ctx.enter_context(tc.tile_pool(name="e", bufs=3))
        empool = ctx.enter_context(tc.tile_pool(name="em", bufs=3))
        spool = ctx.enter_context(tc.tile_pool(name="sm", bufs=3))
        bpool = ctx.enter_context(tc.tile_pool(name="bsb", bufs=3))
        qpool = ctx.enter_context(tc.tile_pool(name="quad", bufs=3))
        rpool = ctx.enter_context(tc.tile_pool(name="rec", bufs=1))
        dpool = ctx.enter_context(tc.tile_pool(name="dram", bufs=1, space="DRAM"))
        ps_em = ctx.enter_context(tc.tile_pool(name="psem", bufs=4, space="PSUM"))
        ps_rep = ctx.enter_context(tc.tile_pool(name="psrep", bufs=1, space="PSUM"))
        ps_b = ctx.enter_context(tc.tile_pool(name="psb", bufs=2, space="PSUM"))

        # ---- constants into SBUF (already target dtype on host) ----
        w_sb = const.tile([128, HC * WP], em_dt)
        nc.gpsimd.dma_start(w_sb[:], w_d[:])
        g4r_sb = const.tile([LL, LL], bf16)
        nc.gpsimd.dma_start(g4r_sb[:], g4r_d[:])
        g4c_sb = const.tile([LL, LL], bf16)
        nc.gpsimd.dma_start(g4c_sb[:], g4c_d[:])
        ra_sb = const.tile([L, LL], bf16)
        nc.gpsimd.dma_start(ra_sb[:], ra_d[:])
        rb_sb = const.tile([L, LL], bf16)
        nc.gpsimd.dma_start(rb_sb[:], rb_d[:])
        # ---- persistent recurrence state ----
        s_tile = rpool.tile([128, LL], bf16)           # chunk-product state
        bc_tile = rpool.tile([128, SPC * LL], bf16)    # quad matrices, chunk layout
        tmp729 = rpool.tile([128, L * L * L], bf16)
        mvals = rpool.tile([128, NNORM], f32)          # applied reciprocal scales

        # DRAM bounce for the quad-layout -> chunk-layout regroup (bf16)
        b_all = dpool.tile([BPC, NQUAD, LL], bf16)

        SP = S + 3  # e_sb column pad (pair col index reaches S; keep 4B align)

        # prefetch all X up front so TensorE never waits on HBM
        xts = []
        for b in range(BPC):
            xt = xpool.tile([128, HC * S], em_dt)
            if b == 0:
                # split so the first matmul can start after half the DMA
                nc.sync.dma_start(xt[:, 0 : 2 * S], xT_d[b, :, 0 : 2 * S])
                nc.sync.dma_start(xt[:, 2 * S : 6 * S], xT_d[b, :, 2 * S : 6 * S])
            else:
                nc.sync.dma_start(xt[:], xT_d[b])
            xts.append(xt)

        def emissions(b, e2, q):
            """Emissions matmul + em download + exp for sequence b.
            Writes exp(em) into half q of the shared pair tile e2."""
            xt = xts[b]
            em_ps = ps_em.tile([L, S], f32)
            if EM_FP8:
                for c in range(HC // 2):
                    nc.tensor.matmul(
                        em_ps[:],
                        w_sb[:, 2 * c * WP : (2 * c + 2) * WP].rearrange(
                            "k (t l) -> k t l", t=2
                        )[:, :, 0:L],
                        xt[:, 2 * c * S : (2 * c + 2) * S].rearrange(
                            "k (t s) -> k t s", t=2
                        ),
                        start=(c == 0),
                        stop=(c == HC // 2 - 1),
                        perf_mode=mybir.MatmulPerfMode.DoubleRow,
                    )
            else:
                for c in range(HC):
                    nc.tensor.matmul(
                        em_ps[:],
                        w_sb[:, c * L : (c + 1) * L],
                        xt[:, c * S : (c + 1) * S],
                        start=(c == 0),
                        stop=(c == HC - 1),
                    )

            # raw emissions download (host computes numerator + v0 + tail)
            em_bf = empool.tile([L, S], bf16)
            nc.scalar.copy(em_bf[:], em_ps[:])
            nc.gpsimd.dma_start(em_out[b], em_bf[:])

            # E = exp(em) in bf16, with a zero column at index S
            nc.vector.memset(e2[:, q, S:SP], 0.0)
            nc.scalar.activation(
                e2[:, q, 0:S], em_ps[:], Act.Exp,
                scale=1.0 / WSCALE if EM_FP8 else 1.0,
            )

        def pair_block(b, e2):
            """Pair matrices for sequences b, b+1 (one batched replication)."""
            # both sequences' E columns in one moving operand [9, 2, 256]
            pstride = e2[:].ap[0][0]
            off = e2[:].offset
            ea_ap = bass.AP(
                e2.tensor, off + 1, [[pstride, L], [SP, 2], [2, NPAIR]]
            )
            eb_ap = bass.AP(
                e2.tensor, off + 2, [[pstride, L], [SP, 2], [2, NPAIR]]
            )
            earep = ps_rep.tile([LL, 2 * NPAIR], f32)
            nc.tensor.matmul(earep[:], ra_sb[:], ea_ap, start=True, stop=True)
            ebrep = ps_rep.tile([LL, 2 * NPAIR], f32)
            nc.tensor.matmul(ebrep[:], rb_sb[:], eb_ap, start=True, stop=True)
            # one PSUM->SBUF copy, then outer = Ea*Eb (one PSUM read allowed)
            ebcp = spool.tile([LL, 2 * NPAIR], bf16)
            nc.scalar.copy(ebcp[:], ebrep[:])
            outer = spool.tile([LL, 2 * NPAIR], bf16)
            nc.vector.tensor_mul(outer[:], earep[:], ebcp[:])

            o_t = outer.tensor
            o_off = outer[:].offset
            o_ps = outer[:].ap[0][0]
            last = b == BPC - 2
            for q in range(2):          # sequence within the pair
                tail = last and q == 1
                bsb = bpool.tile([128, 2 * LL], bf16)
                # pair 255 (odd slot of partition 127) is the zero filler; it
                # must be identity so quad 127 = B_254.  Disjoint region, so
                # this DMA issues immediately and never blocks the merge.
                nc.gpsimd.dma_start(bsb[127:128, LL : 2 * LL], id_d[0:1, :])
                for h in range(2):      # h=0: even pairs (row-major B),
                    bp = ps_b.tile([128, LL], f32)   # h=1: odd (col-major)
                    ocols = bass.AP(
                        o_t, o_off + q * NPAIR + h, [[o_ps, LL], [2, 128]]
                    )
                    nc.tensor.matmul(
                        bp[:], ocols, (g4r_sb if h == 0 else g4c_sb)[:],
                        start=True, stop=True,
                    )
                    nr = 128 if h == 0 else 127
                    nc.vector.tensor_copy(
                        bsb[0:nr, h * LL : (h + 1) * LL], bp[0:nr, :]
                    )
                # quad merge: Q_p = B_{2p} @ B_{2p+1}, emitted col-major
                in0 = (
                    bsb[:, 0:LL].rearrange("p (i k) -> p i k", i=L)
                    .unsqueeze(1).broadcast_to([128, L, L, L])
                )
                in1 = (
                    bsb[:, LL : 2 * LL].rearrange("p (j k) -> p j k", j=L)
                    .unsqueeze(2).broadcast_to([128, L, L, L])
                )
                t3 = tmp729[:].rearrange("p (j i k) -> p j i k", j=L, i=L)
                nc.vector.tensor_tensor(out=t3, in0=in0, in1=in1, op=Alu.mult)
                qsb = qpool.tile([128, LL], bf16)
                with nc.allow_low_precision(reason="host chains in f64"):
                    nc.vector.tensor_reduce(
                        out=qsb[:], in_=t3, axis=Ax.X, op=Alu.add
                    )
                # bounce: quad rows out, chunk-layout read back
                rd = b_all[b + q].rearrange("(c s) j -> c (s j)", c=NCHUNK)
                rows = bc_tile[16 * (b + q) : 16 * (b + q + 1), :]
                nc.scalar.dma_start(b_all[b + q], qsb[:])
                if tail:
                    # early slots first so the recurrence can start sooner
                    nc.sync.dma_start(rows[:, 0 : 3 * LL], rd[:, 0 : 3 * LL])
                    nc.sync.dma_start(rows[:, 3 * LL :], rd[:, 3 * LL :])
                elif last:
                    nc.sync.dma_start(rows, rd)
                else:
                    nc.gpsimd.dma_start(rows, rd)

        # run emissions well ahead of the pair blocks: TensorE stays dense
        # (no LOW-p-state restarts) and rep/pair matmuls never wait on exp
        e2s = {}
        for b in range(BPC):
            if b % 2 == 0:
                e2 = epool.tile([L, 2, SP], bf16, name=f"e2_{b}")
                e2s[b] = e2
            emissions(b, e2s[b - b % 2], b % 2)
            if b == 3:
                pair_block(0, e2s[0])
            elif b == 5:
                pair_block(2, e2s[2])
            elif b == 7:
                pair_block(4, e2s[4])
        pair_block(6, e2s[6])

        # ---- chunk-parallel matrix recurrence: S <- S @ Q_s (bf16) ----
        # init: S = Q_0 (stored col-major; transpose-copy to row-major)
        nc.vector.tensor_copy(
            s_tile[:].rearrange("p (i j) -> p i j", i=L),
            bc_tile[:, 0:LL].rearrange("p (j i) -> p i j", j=L),
        )
        ncol = 0
        for s in range(1, SPC):
            bs = bc_tile[:, s * LL : (s + 1) * LL]
            in0 = (
                s_tile[:].rearrange("p (i k) -> p i k", i=L)
                .unsqueeze(2).broadcast_to([128, L, L, L])
            )
            # bc stores B^T (column-major B): inner k is contiguous
            in1 = (
                bs.rearrange("p (j k) -> p j k", j=L)
                .unsqueeze(1).broadcast_to([128, L, L, L])
            )
            t3 = tmp729[:].rearrange("p (i j k) -> p i j k", i=L, j=L)
            nc.vector.tensor_tensor(out=t3, in0=in0, in1=in1, op=Alu.mult)
            with nc.allow_low_precision(reason="9-term sums; host chains in f64"):
                nc.vector.tensor_reduce(
                    out=s_tile[:], in_=t3, axis=Ax.X, op=Alu.add
                )
            if s in NORM_STEPS:
                mc = spool.tile([128, 1], f32)
                nc.vector.reduce_max(mc[:], s_tile[:], axis=Ax.X)
                rec = mvals[:, ncol : ncol + 1]
                ncol += 1
                nc.vector.reciprocal(rec, mc[:])
                nc.vector.tensor_scalar_mul(s_tile[:], s_tile[:], rec)

        nc.sync.dma_start(s_out[:], s_tile[:])
        nc.sync.dma_start(m_out[:], mvals[:])

    if not nc.is_finalized():
        nc.finalize()
    return nc


def _get_nc():
    if "nc" not in _CACHE:
        _CACHE["nc"] = _build_bass()
    return _CACHE["nc"]


def _host_consts(trans):
    import ml_dtypes

    bf = ml_dtypes.bfloat16
    expT = np.exp(trans.astype(np.float64)).astype(np.float32)  # [9,9]
    k_idx = np.arange(LL) // L   # row index of the 81-flat (k, jb)
    jb_idx = np.arange(LL) % L
    i_idx = np.arange(LL) // L   # col index of the 81-flat (i, j)
    j_idx = np.arange(LL) % L
    # G4[(k,jb),(i,j)] = expT[i,k] * expT[k,j] * (j == jb)
    g4 = (
        expT[np.ix_(i_idx, k_idx)].T
        * expT[np.ix_(k_idx, j_idx)]
        * (j_idx[None, :] == jb_idx[:, None])
    ).astype(np.float32)
    g4r = np.ascontiguousarray(g4).astype(bf)       # row-major B (even pairs)
    # column-major B (odd pairs): contiguous reads in the quad merge
    g4c = np.ascontiguousarray(
        g4.reshape(LL, L, L).swapaxes(1, 2).reshape(LL, LL)
    ).astype(bf)
    ra = (k_idx[None, :] == np.arange(L)[:, None]).astype(bf)   # [9,81]
    rb = (jb_idx[None, :] == np.arange(L)[:, None]).astype(bf)  # [9,81]
    id128 = np.tile(
        np.eye(L, dtype=np.float32).reshape(1, LL), (128, 1)
    ).astype(bf)
    return expT, g4r, g4c, ra, rb, id128


def _numpy_reference(hs, mask, labels, W, bb, st, en, tr):
    # general fallback (only used when attention_mask is not all ones)
    em = hs.astype(np.float64) @ W.astype(np.float64) + bb.astype(np.float64)
    maskb = mask.astype(bool)
    maskf = mask.astype(np.float64)
    em_tag = np.take_along_axis(em, labels[..., None], axis=-1)[..., 0]
    num = st.astype(np.float64)[labels[:, 0]] + em_tag[:, 0]
    trs = tr.astype(np.float64)[labels[:, :-1], labels[:, 1:]]
    num = num + np.sum((trs + em_tag[:, 1:]) * maskf[:, 1:], axis=1)
    last = mask.sum(axis=1).astype(np.int64) - 1
    num = num + en.astype(np.float64)[labels[np.arange(len(labels)), last]]
    alpha = st.astype(np.float64)[None, :] + em[:, 0]
    for t in range(1, em.shape[1]):
        x = alpha[:, :, None] + tr.astype(np.float64)[None, :, :] + em[:, t][:, None, :]
        m = x.max(axis=1, keepdims=True)
        nxt = np.log(np.exp(x - m).sum(axis=1)) + m[:, 0, :]
        alpha = np.where(maskb[:, t][:, None], nxt, alpha)
    x = alpha + en.astype(np.float64)[None, :]
    m = x.max(axis=1, keepdims=True)
    denom = np.log(np.exp(x - m).sum(axis=1)) + m[:, 0]
    return np.asarray((denom - num).sum(), dtype=np.float32)


def kernel(**inputs):
    import ml_dtypes
    from concourse import bass_utils

    hs = np.asarray(inputs["hidden_states"], dtype=np.float32)
    mask = np.asarray(inputs["attention_mask"])
    labels = np.asarray(inputs["labels"]).astype(np.int64)
    W = np.asarray(inputs["W"], dtype=np.float32)
    bb = np.asarray(inputs["b"], dtype=np.float32)
    st = np.asarray(inputs["start_trans"], dtype=np.float32)
    en = np.asarray(inputs["end_trans"], dtype=np.float32)
    tr = np.asarray(inputs["trans"], dtype=np.float32)

    if not np.all(mask == 1):
        return _numpy_reference(hs, mask, labels, W, bb, st, en, tr)

    em_np = ml_dtypes.float8_e4m3 if EM_FP8 else ml_dtypes.bfloat16
    expT, g4r, g4c, ra, rb, id128 = _host_consts(tr)

    # X^T in matmul layout: [B, 128, HC*S], partition k holds H rows c*128+k
    if EM_FP8:
        xc = hs.astype(em_np)
    else:
        xc = hs.astype(em_np)
    xT = np.ascontiguousarray(
        xc.reshape(B, S, HC, 128).transpose(0, 3, 2, 1)
    ).reshape(B, 128, HC * S)
    ws = (W * WSCALE) if EM_FP8 else W
    wT = np.ascontiguousarray(
        ws.reshape(HC, 128, L).transpose(1, 0, 2)
    ).astype(em_np)                                   # [128, HC, L]
    if EM_FP8:
        wp = np.zeros((128, HC, 16), dtype=em_np)
        wp[:, :, :L] = wT
        wT = wp
    wT = wT.reshape(128, -1)

    nc = _get_nc()
    in_maps = []
    for k in range(NCORES):
        sl = slice(k * BPC, (k + 1) * BPC)
        in_maps.append(
            {
                "xT": xT[sl],
                "Wt": wT,
                "G4R": g4r,
                "G4C": g4c,
                "Ra": ra,
                "Rb": rb,
                "Id128": id128,
            }
        )
    res = bass_utils.run_bass_kernel_spmd(nc, in_maps, list(range(NCORES)))
    _CACHE["last_results"] = res

    # ---- host combine (f64, tiny) ----
    expT64 = np.exp(tr.astype(np.float64))
    e_end = np.exp(en.astype(np.float64))
    st64 = st.astype(np.float64)
    bb64 = bb.astype(np.float64)
    en64 = en.astype(np.float64)
    tr64 = tr.astype(np.float64)
    total = 0.0
    for k in range(NCORES):
        r = res.results[k]
        em = r["em_out"].astype(np.float64)          # [BPC, 9, S]
        if EM_FP8:
            em = em / WSCALE
        Sf = r["S_out"].astype(np.float64).reshape(BPC, NCHUNK, L, L)
        mv = r["m_out"].astype(np.float64).reshape(BPC, NCHUNK, NNORM)
        for b in range(BPC):
            v = np.exp(em[b, :, 0] + st64 + bb64)    # v0
            logacc = -np.log(mv[b]).sum()            # undo applied scales
            for c in range(NCHUNK):
                v = v @ Sf[b, c]
                m = v.max()
                v /= m
                logacc += np.log(m)
            v = (v @ expT64) * np.exp(em[b, :, S - 1] + bb64)  # tail t = S-1
            total += np.log(v @ e_end) + logacc
        # numerator for this core's sequences (gold path score)
        lb = labels[k * BPC : (k + 1) * BPC]
        em_tag = np.take_along_axis(em, lb[:, None, :], axis=1)[:, 0, :]  # [BPC,S]
        total -= float(
            em_tag.sum()
            + st64[lb[:, 0]].sum()
            + en64[lb[:, -1]].sum()
            + tr64[lb[:, :-1], lb[:, 1:]].sum()
            + bb64[lb].sum()
        )
    return np.asarray(total, dtype=np.float32)
